# revision 4
# baseline (speedup 1.0000x reference)
"""Trainium2 Bass kernel for the EighMSE loss (data-parallel over 8 cores).

Math (replicates jax/LAPACK ssyevd eigenvector sign conventions for 2x2
symmetric matrices; see derivation in comments at bottom):
  row (a, b, c) encodes [[a, b], [b, c]]
  SM = a + c, DF = a - c, RT = sqrt(DF^2 + 4 b^2)
  closed-form evals = (SM +- RT) / 2
  x = clamp(DF / RT, -1, 1)
  n0 = sqrt((1 + x)/2) >= 0,  n1u = sqrt((1 - x)/2) >= 0
  LAPACK larger-eval eigenvector g = (tau0 * n0, tau1 * n1u) with
    tau0 = -1 if DF > 0 else sign(b) * sign(SM)
    tau1 = tau0 * sign(b)
  smaller-eval eigenvector = sign(SM) * (-g1, g0)

Per-core partial sums (10 f32 values per partition, summed on host):
  A  = sum dSM^2      Bs = sum dRT^2     C = sum dDF^2     D = sum db^2
  E1 = sum dg0^2      E2 = sum dg1^2
  SP0 = sum g0p*g0t   SP1 = sum g1p*g1t
  SP0m = sum sx*g0p*g0t   SP1m = sum sx*g1p*g1t   (sx = [sign(SMp) != sign(SMt)])
Host combine:
  F0 = E1 + 4*SP0m ; F1 = E2 + 4*SP1m   (the sign(SM)-flipped evec diffs)
  loss = w0*(A+Bs)/(4B) + w1*E1/B + w2*E2/B + w3*F1/B + w4*F0/B
         + w5*(A/2 + C/2 + D)/(3B)
"""

import numpy as np
from contextlib import ExitStack

import concourse.bass as bass
import concourse.bacc as bacc
import concourse.tile as tile
from concourse import mybir
from concourse.bass_utils import run_bass_kernel_spmd

F32 = mybir.dt.float32
BF16 = mybir.dt.bfloat16
OP = mybir.AluOpType
AF = mybir.ActivationFunctionType

B_TOTAL = 4_194_304
NCORES = 8
S = B_TOTAL // NCORES          # samples per core
P = 128                        # partitions
NPC = S // P                   # samples per partition (4096)
W = 512                        # samples per tile per partition
NT = NPC // W                  # tiles per core
NTERMS = 10

_BUILT = None


def _build_bass():
    nc = bacc.Bacc()
    yp = nc.declare_dram_parameter("y_pred", [S, 3], F32, isOutput=False)
    yt = nc.declare_dram_parameter("y_true", [S, 3], F32, isOutput=False)
    out = nc.declare_dram_parameter("out", [P, NTERMS], F32, isOutput=True)

    ypr = yp.rearrange("(p n) c -> p n c", p=P)
    ytr = yt.rearrange("(p n) c -> p n c", p=P)

    with tile.TileContext(nc) as tc, ExitStack() as ctx:
        inp = ctx.enter_context(tc.tile_pool(name="inp", bufs=2))
        wk = ctx.enter_context(tc.tile_pool(name="wk", bufs=2))
        bk = ctx.enter_context(tc.tile_pool(name="bk", bufs=1))
        accp = ctx.enter_context(tc.tile_pool(name="accp", bufs=1))

        stats = accp.tile([P, NTERMS * NT], F32)
        nc.vector.memset(stats[:], 0.0)
        halfc = accp.tile([P, 1], F32)
        nc.vector.memset(halfc[:], 0.5)

        def tensor_chain(x, pref):
            a = x[:, :, 0]
            b = x[:, :, 1]
            c = x[:, :, 2]

            SM = wk.tile([P, W], F32, tag=pref + "SM")
            nc.vector.tensor_add(SM[:], a, c)
            DF = wk.tile([P, W], F32, tag=pref + "DF")
            nc.vector.tensor_sub(DF[:], a, c)

            TB2 = wk.tile([P, W], F32, tag=pref + "TB2")
            nc.scalar.activation(TB2[:], b, AF.Square, scale=2.0)
            DF2 = wk.tile([P, W], F32, tag=pref + "DF2")
            nc.scalar.activation(DF2[:], DF[:], AF.Square)
            RT2 = wk.tile([P, W], F32, tag=pref + "RT2")
            nc.vector.tensor_add(RT2[:], DF2[:], TB2[:])
            RT = wk.tile([P, W], F32, tag=pref + "RT")
            nc.scalar.activation(RT[:], RT2[:], AF.Sqrt)

            r = wk.tile([P, W], F32, tag=pref + "r")
            nc.vector.reciprocal_approx_fast(r[:], RT[:])
            x_ = wk.tile([P, W], F32, tag=pref + "x")
            nc.vector.tensor_mul(x_[:], DF[:], r[:])
            xc = wk.tile([P, W], F32, tag=pref + "xc")
            nc.vector.tensor_scalar(xc[:], x_[:], 1.0, -1.0, op0=OP.min, op1=OP.max)

            n0 = bk.tile([P, W], BF16, tag=pref + "n0")
            nc.scalar.activation(n0[:], xc[:], AF.Sqrt, bias=halfc[:], scale=0.5)
            n1u = bk.tile([P, W], BF16, tag=pref + "n1u")
            nc.scalar.activation(n1u[:], xc[:], AF.Sqrt, bias=halfc[:], scale=-0.5)

            # masks (1/0 in bf16): mb = b<0, ms = SM<0, mDF = DF>0
            mb = bk.tile([P, W], BF16, tag=pref + "mb")
            nc.vector.tensor_single_scalar(mb[:], b, 0.0, op=OP.is_lt)
            ms = bk.tile([P, W], BF16, tag=pref + "ms")
            nc.vector.tensor_single_scalar(ms[:], SM[:], 0.0, op=OP.is_lt)
            mDF = bk.tile([P, W], BF16, tag=pref + "mDF")
            nc.vector.tensor_single_scalar(mDF[:], DF[:], 0.0, op=OP.is_gt)

            # q0 = [tau0 < 0] = mDF OR (mb XOR ms) ; q1 = q0 XOR mb
            mneg = bk.tile([P, W], BF16, tag=pref + "mneg")
            nc.vector.tensor_tensor(mneg[:], mb[:], ms[:], op=OP.not_equal)
            q0 = bk.tile([P, W], BF16, tag=pref + "q0")
            nc.vector.tensor_max(q0[:], mDF[:], mneg[:])
            q1 = bk.tile([P, W], BF16, tag=pref + "q1")
            nc.vector.tensor_tensor(q1[:], q0[:], mb[:], op=OP.not_equal)

            t0 = bk.tile([P, W], BF16, tag=pref + "t0")
            nc.vector.tensor_scalar(t0[:], q0[:], -2.0, 1.0, op0=OP.mult, op1=OP.add)
            t1 = bk.tile([P, W], BF16, tag=pref + "t1")
            nc.vector.tensor_scalar(t1[:], q1[:], -2.0, 1.0, op0=OP.mult, op1=OP.add)

            g0 = bk.tile([P, W], BF16, tag=pref + "g0")
            nc.vector.tensor_mul(g0[:], t0[:], n0[:])
            g1 = bk.tile([P, W], BF16, tag=pref + "g1")
            nc.vector.tensor_mul(g1[:], t1[:], n1u[:])

            return dict(SM=SM[:], DF=DF[:], RT=RT[:], b=b, g0=g0[:], g1=g1[:], ms=ms[:])

        scr = accp.tile([P, W], F32)

        def sq_acc(d_ap, col):
            nc.scalar.activation(scr[:], d_ap, AF.Square, accum_out=stats[:, col : col + 1])

        def cp_acc(d_ap, col):
            nc.scalar.activation(scr[:], d_ap, AF.Copy, accum_out=stats[:, col : col + 1])

        for i in range(NT):
            xp = inp.tile([P, W, 3], F32, tag="xp")
            nc.sync.dma_start(xp[:], ypr[:, bass.ts(i, W), :])
            xt = inp.tile([P, W, 3], F32, tag="xt")
            nc.sync.dma_start(xt[:], ytr[:, bass.ts(i, W), :])

            tp = tensor_chain(xp, "p_")
            tt = tensor_chain(xt, "t_")

            # linear / eigenvalue diff terms (f32)
            for k, name in enumerate(["SM", "RT", "DF", "b"]):
                d = wk.tile([P, W], F32, tag="d_" + name)
                nc.vector.tensor_sub(d[:], tp[name], tt[name])
                sq_acc(d[:], k * NT + i)

            # eigenvector diff terms (bf16)
            dg0 = bk.tile([P, W], BF16, tag="dg0")
            nc.vector.tensor_sub(dg0[:], tp["g0"], tt["g0"])
            sq_acc(dg0[:], 4 * NT + i)
            dg1 = bk.tile([P, W], BF16, tag="dg1")
            nc.vector.tensor_sub(dg1[:], tp["g1"], tt["g1"])
            sq_acc(dg1[:], 5 * NT + i)

            # cross products for the sign(SM)-flipped terms
            sx = bk.tile([P, W], BF16, tag="sx")
            nc.vector.tensor_tensor(sx[:], tp["ms"], tt["ms"], op=OP.not_equal)
            P0 = bk.tile([P, W], BF16, tag="P0")
            nc.vector.tensor_mul(P0[:], tp["g0"], tt["g0"])
            P1 = bk.tile([P, W], BF16, tag="P1")
            nc.vector.tensor_mul(P1[:], tp["g1"], tt["g1"])
            P0m = bk.tile([P, W], BF16, tag="P0m")
            nc.vector.tensor_mul(P0m[:], P0[:], sx[:])
            P1m = bk.tile([P, W], BF16, tag="P1m")
            nc.vector.tensor_mul(P1m[:], P1[:], sx[:])
            cp_acc(P0[:], 6 * NT + i)
            cp_acc(P1[:], 7 * NT + i)
            cp_acc(P0m[:], 8 * NT + i)
            cp_acc(P1m[:], 9 * NT + i)

        outsums = accp.tile([P, NTERMS], F32)
        stats3 = stats[:].rearrange("p (t i) -> p t i", t=NTERMS)
        for t in range(NTERMS):
            nc.vector.tensor_reduce(
                outsums[:, t : t + 1], stats3[:, t, :], axis=mybir.AxisListType.X, op=OP.add
            )
        nc.sync.dma_start(out[:, :], outsums[:])

    nc.compile()
    return nc


def _get_built():
    global _BUILT
    if _BUILT is None:
        _BUILT = _build_bass()
    return _BUILT


def kernel(y_pred: np.ndarray, y_true: np.ndarray, weights: np.ndarray) -> np.ndarray:
    y_pred = np.ascontiguousarray(y_pred, dtype=np.float32)
    y_true = np.ascontiguousarray(y_true, dtype=np.float32)
    w = np.asarray(weights, dtype=np.float64)

    nc = _get_built()
    in_maps = []
    for c in range(NCORES):
        in_maps.append(
            {
                "y_pred": y_pred[c * S : (c + 1) * S],
                "y_true": y_true[c * S : (c + 1) * S],
            }
        )
    res = run_bass_kernel_spmd(nc, in_maps, list(range(NCORES)))
    sums = np.zeros(NTERMS, dtype=np.float64)
    for c in range(NCORES):
        sums += np.asarray(res.results[c]["out"], dtype=np.float64).sum(axis=0)

    A, Bs, C, D, E1, E2, SP0, SP1, SP0m, SP1m = sums
    F0 = E1 + 4.0 * SP0m
    F1 = E2 + 4.0 * SP1m
    Bn = float(B_TOTAL)
    evals_mse = (A + Bs) / (4.0 * Bn)
    mse_loss = (0.5 * A + 0.5 * C + D) / (3.0 * Bn)
    loss = (
        w[0] * evals_mse
        + w[1] * E1 / Bn
        + w[2] * E2 / Bn
        + w[3] * F1 / Bn
        + w[4] * F0 / Bn
        + w[5] * mse_loss
    )
    return np.float32(loss)


# revision 5
# speedup vs baseline: 9975.4657x; 9975.4657x over previous
"""Trainium2 Bass kernel for the EighMSE loss (data-parallel over 8 cores).

Math (replicates jax/LAPACK ssyevd eigenvector sign conventions for 2x2
symmetric matrices):
  row (a, b, c) encodes [[a, b], [b, c]]
  SM = a + c, DF = a - c, RT = sqrt(DF^2 + 4 b^2)
  closed-form evals = (SM +- RT) / 2
  x = clamp(DF / RT, -1, 1)
  n0 = sqrt((1 + x)/2) >= 0,  n1u = sqrt((1 - x)/2) >= 0
  LAPACK larger-eval eigenvector g = (tau0 * n0, tau1 * n1u) with
    tau0 = -1 if DF > 0 else sign(b) * sign(SM)
    tau1 = tau0 * sign(b)
  smaller-eval eigenvector = sign(SM) * (-g1, g0)

Per-core partial sums (10 f32 values per partition, summed on host):
  A  = sum dSM^2      Bs = sum dRT^2     C = sum dDF^2     D = sum db^2
  E1 = sum dg0^2      E2 = sum dg1^2
  SP0 = sum g0p*g0t   SP1 = sum g1p*g1t
  SP0m = sum sx*g0p*g0t   SP1m = sum sx*g1p*g1t  (sx = [sign(SMp) != sign(SMt)])
Host combine:
  F0 = E1 + 4*SP0m ; F1 = E2 + 4*SP1m
  loss = w0*(A+Bs)/(4B) + w1*E1/B + w2*E2/B + w3*F1/B + w4*F0/B
         + w5*(A/2 + C/2 + D)/(3B)
"""

import numpy as np
from contextlib import ExitStack

import concourse.bass as bass
import concourse.bacc as bacc
import concourse.tile as tile
from concourse import mybir
from concourse.bass_utils import run_bass_kernel_spmd

F32 = mybir.dt.float32
BF16 = mybir.dt.bfloat16
OP = mybir.AluOpType
AF = mybir.ActivationFunctionType

B_TOTAL = 4_194_304
NCORES = 8
S = B_TOTAL // NCORES          # samples per core
P = 128                        # partitions
NPC = S // P                   # samples per partition (4096)
W = 1024                       # samples per tile per partition
NT = NPC // W                  # tiles per core
NTERMS = 10

_BUILT = None


def _build_bass():
    nc = bacc.Bacc()
    yp = nc.declare_dram_parameter("y_pred", [S, 3], F32, isOutput=False)
    yt = nc.declare_dram_parameter("y_true", [S, 3], F32, isOutput=False)
    out = nc.declare_dram_parameter("out", [P, NTERMS], F32, isOutput=True)

    ypr = yp.rearrange("(p n) c -> p n c", p=P)
    ytr = yt.rearrange("(p n) c -> p n c", p=P)

    with tile.TileContext(nc) as tc, ExitStack() as ctx:
        inp = ctx.enter_context(tc.tile_pool(name="inp", bufs=2))
        wk = ctx.enter_context(tc.tile_pool(name="wk", bufs=1))
        bk = ctx.enter_context(tc.tile_pool(name="bk", bufs=1))
        dp = ctx.enter_context(tc.tile_pool(name="dp", bufs=2))
        accp = ctx.enter_context(tc.tile_pool(name="accp", bufs=1))

        stats = accp.tile([P, NTERMS * NT], F32)
        nc.vector.memset(stats[:], 0.0)
        halfc = accp.tile([P, 1], F32)
        nc.vector.memset(halfc[:], 0.5)

        def tensor_chain(x, pref):
            a = x[:, :, 0]
            b = x[:, :, 1]
            c = x[:, :, 2]

            SM = wk.tile([P, W], F32, tag=pref + "SM")
            nc.vector.tensor_add(SM[:], a, c)
            DF = wk.tile([P, W], F32, tag=pref + "DF")
            nc.vector.tensor_sub(DF[:], a, c)

            sq1 = wk.tile([P, W], F32, tag=pref + "sq1")   # TB2 -> RT2
            nc.scalar.activation(sq1[:], b, AF.Square, scale=2.0)
            sq2 = wk.tile([P, W], F32, tag=pref + "sq2")   # DF2 -> RT
            nc.scalar.activation(sq2[:], DF[:], AF.Square)
            nc.vector.tensor_add(sq1[:], sq2[:], sq1[:])          # RT2 (in-place)
            nc.scalar.activation(sq2[:], sq1[:], AF.Sqrt)         # RT
            RT = sq2

            r = wk.tile([P, W], F32, tag=pref + "r")       # r -> x -> xc
            nc.vector.reciprocal_approx_fast(r[:], RT[:])
            nc.vector.tensor_mul(r[:], DF[:], r[:])               # x (in-place)
            nc.vector.tensor_scalar(r[:], r[:], 1.0, -1.0, op0=OP.min, op1=OP.max)
            xc = r

            n0 = bk.tile([P, W], BF16, tag=pref + "n0")
            nc.scalar.activation(n0[:], xc[:], AF.Sqrt, bias=halfc[:], scale=0.5)
            n1u = bk.tile([P, W], BF16, tag=pref + "n1u")
            nc.scalar.activation(n1u[:], xc[:], AF.Sqrt, bias=halfc[:], scale=-0.5)

            # masks (1/0 in bf16): mb = b<0, ms = SM<0, mDF = DF>0
            mb = bk.tile([P, W], BF16, tag=pref + "mb")    # mb -> q1 -> t1 -> g1
            nc.vector.tensor_single_scalar(mb[:], b, 0.0, op=OP.is_lt)
            ms = bk.tile([P, W], BF16, tag=pref + "ms")
            nc.vector.tensor_single_scalar(ms[:], SM[:], 0.0, op=OP.is_lt)
            mDF = bk.tile([P, W], BF16, tag=pref + "mDF")
            nc.vector.tensor_single_scalar(mDF[:], DF[:], 0.0, op=OP.is_gt)

            # q0 = [tau0<0] = mDF OR (mb XOR ms) ; q1 = q0 XOR mb
            mg = bk.tile([P, W], BF16, tag=pref + "mg")    # mneg -> q0 -> t0 -> g0
            nc.vector.tensor_tensor(mg[:], mb[:], ms[:], op=OP.not_equal)
            nc.vector.tensor_max(mg[:], mDF[:], mg[:])            # q0 (in-place)
            nc.vector.tensor_tensor(mb[:], mg[:], mb[:], op=OP.not_equal)  # q1
            # tau = 1 - 2q
            nc.vector.tensor_scalar(mg[:], mg[:], -2.0, 1.0, op0=OP.mult, op1=OP.add)
            nc.vector.tensor_scalar(mb[:], mb[:], -2.0, 1.0, op0=OP.mult, op1=OP.add)
            # g = tau * n
            nc.vector.tensor_mul(mg[:], mg[:], n0[:])             # g0
            nc.vector.tensor_mul(mb[:], mb[:], n1u[:])            # g1

            return dict(SM=SM[:], RT=RT[:], DF=DF[:], b=b, g0=mg[:], g1=mb[:], ms=ms[:])

        scr = accp.tile([P, W], F32)

        def sq_acc(d_ap, col):
            nc.scalar.activation(scr[:], d_ap, AF.Square, accum_out=stats[:, col : col + 1])

        def cp_acc(d_ap, col):
            nc.scalar.activation(scr[:], d_ap, AF.Copy, accum_out=stats[:, col : col + 1])

        for i in range(NT):
            xp = inp.tile([P, W, 3], F32, tag="xp")
            nc.sync.dma_start(xp[:], ypr[:, bass.ts(i, W), :])
            xt = inp.tile([P, W, 3], F32, tag="xt")
            nc.sync.dma_start(xt[:], ytr[:, bass.ts(i, W), :])

            tp = tensor_chain(xp, "p_")
            tt = tensor_chain(xt, "t_")

            # linear / eigenvalue diff terms (f32)
            for k, name in enumerate(["SM", "RT", "DF", "b"]):
                d = dp.tile([P, W], F32, tag="d")
                nc.vector.tensor_sub(d[:], tp[name], tt[name])
                sq_acc(d[:], k * NT + i)

            # eigenvector diff terms (bf16)
            dg0 = dp.tile([P, W], BF16, tag="dg0")
            nc.vector.tensor_sub(dg0[:], tp["g0"], tt["g0"])
            sq_acc(dg0[:], 4 * NT + i)
            dg1 = dp.tile([P, W], BF16, tag="dg1")
            nc.vector.tensor_sub(dg1[:], tp["g1"], tt["g1"])
            sq_acc(dg1[:], 5 * NT + i)

            # cross products for the sign(SM)-flipped terms
            sx = dp.tile([P, W], BF16, tag="sx")
            nc.vector.tensor_tensor(sx[:], tp["ms"], tt["ms"], op=OP.not_equal)
            P0 = dp.tile([P, W], BF16, tag="P0")
            nc.vector.tensor_mul(P0[:], tp["g0"], tt["g0"])
            P1 = dp.tile([P, W], BF16, tag="P1")
            nc.vector.tensor_mul(P1[:], tp["g1"], tt["g1"])
            P0m = dp.tile([P, W], BF16, tag="P0m")
            nc.vector.tensor_mul(P0m[:], P0[:], sx[:])
            P1m = dp.tile([P, W], BF16, tag="P1m")
            nc.vector.tensor_mul(P1m[:], P1[:], sx[:])
            cp_acc(P0[:], 6 * NT + i)
            cp_acc(P1[:], 7 * NT + i)
            cp_acc(P0m[:], 8 * NT + i)
            cp_acc(P1m[:], 9 * NT + i)

        outsums = accp.tile([P, NTERMS], F32)
        stats3 = stats[:].rearrange("p (t i) -> p t i", t=NTERMS)
        for t in range(NTERMS):
            nc.vector.tensor_reduce(
                outsums[:, t : t + 1], stats3[:, t, :], axis=mybir.AxisListType.X, op=OP.add
            )
        nc.sync.dma_start(out[:, :], outsums[:])

    nc.compile()
    return nc


def _get_built():
    global _BUILT
    if _BUILT is None:
        _BUILT = _build_bass()
    return _BUILT


def kernel(y_pred: np.ndarray, y_true: np.ndarray, weights: np.ndarray) -> np.ndarray:
    y_pred = np.ascontiguousarray(y_pred, dtype=np.float32)
    y_true = np.ascontiguousarray(y_true, dtype=np.float32)
    w = np.asarray(weights, dtype=np.float64)

    nc = _get_built()
    in_maps = []
    for c in range(NCORES):
        in_maps.append(
            {
                "y_pred": y_pred[c * S : (c + 1) * S],
                "y_true": y_true[c * S : (c + 1) * S],
            }
        )
    res = run_bass_kernel_spmd(nc, in_maps, list(range(NCORES)))
    sums = np.zeros(NTERMS, dtype=np.float64)
    for c in range(NCORES):
        sums += np.asarray(res.results[c]["out"], dtype=np.float64).sum(axis=0)

    A, Bs, C, D, E1, E2, SP0, SP1, SP0m, SP1m = sums
    F0 = E1 + 4.0 * SP0m
    F1 = E2 + 4.0 * SP1m
    Bn = float(B_TOTAL)
    evals_mse = (A + Bs) / (4.0 * Bn)
    mse_loss = (0.5 * A + 0.5 * C + D) / (3.0 * Bn)
    loss = (
        w[0] * evals_mse
        + w[1] * E1 / Bn
        + w[2] * E2 / Bn
        + w[3] * F1 / Bn
        + w[4] * F0 / Bn
        + w[5] * mse_loss
    )
    return np.float32(loss)
